# revision 6
# baseline (speedup 1.0000x reference)
"""Trainium2 Bass kernel for nn_MemoryAggregator (GNN attention aggregation).

Reference computation:
    Q = X@Wq; K = X@Wk; V = X@Wv            (X [100000,256], W [256,32])
    scores_e = <Q[src_e], K[dst_e]> / sqrt(32)   over 1.6M edges
    out[n]   = softmax-weighted sum over n's edges of V[dst_e]   ([100000,32])

Strategy (8 NeuronCores, SPMD, edges sharded by src node range):
  kernel1: per-core QKV projections of the core's 12500-node X shard (PE matmul).
  host:    arrange per-edge K|V rows and per-pair Q rows into flat per-partition
           slot streams (bf16).  Pad slots get K = -C*q/|q|^2 so their score is
           ~-35 and exp underflows to 0 -- no mask stream needed.
  kernel2: per core, stream slot blocks sequentially (no gathers) and compute
           scores -> exp -> pair partial sums [num(32) | den] on DVE/ACT.
  host:    per-node segment reduction of pair partials (contiguous runs,
           np.add.reduceat) + division.

Softmax max-subtraction is dropped: scores/sqrt(32) ~ N(0,4), max ~21, exp
safe in f32 (validated earlier at rel err ~3e-6; bf16 streams ~1e-3).
"""
import math
from contextlib import ExitStack

import numpy as np

import concourse.bass as bass
import concourse.tile as tile
from concourse import bacc, mybir
from concourse.bass_utils import run_bass_kernel_spmd

# ---------------------------------------------------------------- dimensions
N = 100000
E = 1600000
D_IN = 256
H = 32
DK = math.sqrt(H)
NCORES = 8
NPC = N // NCORES          # 12500 nodes per core
P = 128
B = 240                    # slot block width per k2 step (even)

BF16 = mybir.dt.np(mybir.dt.bfloat16)

_cache = {}
LAST_TIMES = {}


# ================================================================ host prep
def _prep_core(src_l, dst):
    order = np.argsort(src_l, kind="stable")
    dst_s = dst[order].astype(np.int32)

    d = np.bincount(src_l, minlength=NPC)
    v = (d + 1) // 2
    s = 2 * v

    # partition assignment: serpentine over nodes sorted by size desc
    node_order = np.argsort(-s, kind="stable")
    i = np.arange(NPC)
    pos = i % P
    pserp = np.where((i // P) % 2 == 0, pos, P - 1 - pos)
    part = np.empty(NPC, np.int64)
    part[node_order] = pserp
    load = np.bincount(part, weights=s, minlength=P).astype(np.int64)
    return {"d": d, "v": v, "s": s, "part": part, "dst_s": dst_s,
            "S_core": int(load.max())}


def _finalize_core(cc, S):
    NV = S // 2
    d, v, s, part = cc["d"], cc["v"], cc["s"], cc["part"]

    perm = np.lexsort((np.arange(NPC), part))
    part_sorted = part[perm]
    sizes = s[perm]
    cs = np.cumsum(sizes) - sizes
    pstart = np.searchsorted(part_sorted, np.arange(P))
    base_at = cs[np.minimum(pstart, NPC - 1)]
    within = cs - base_at[part_sorted]
    within_node = np.empty(NPC, np.int64)
    within_node[perm] = within

    slotdst = np.full((P, S), -1, np.int32)
    nodes_rep = np.repeat(np.arange(NPC), d)
    ranks = np.arange(int(d.sum())) - np.repeat(np.cumsum(d) - d, d)
    cols = within_node[nodes_rep] + ranks
    slotdst[part[nodes_rep], cols] = cc["dst_s"]

    qvnode = np.full((P, NV), -1, np.int32)
    vrep = np.repeat(np.arange(NPC), v)
    vranks = np.arange(int(v.sum())) - np.repeat(np.cumsum(v) - v, v)
    vcols = (within_node[vrep] >> 1) + vranks
    qvnode[part[vrep], vcols] = vrep

    cc["slotdst"] = slotdst
    cc["qvnode"] = qvnode
    del cc["dst_s"], cc["d"], cc["v"], cc["s"], cc["part"]


def _prep(edge_index):
    src = np.asarray(edge_index[0], dtype=np.int64)
    dst = np.asarray(edge_index[1], dtype=np.int64)
    core = src // NPC
    cores = []
    for c in range(NCORES):
        m = core == c
        cores.append(_prep_core(src[m] - c * NPC, dst[m]))
    S = max(cc["S_core"] for cc in cores)
    S = (S + 1) // 2 * 2
    for cc in cores:
        _finalize_core(cc, S)
    return cores, S


# ================================================================ kernel 1
K1M = 125                      # rows per matmul tile; 100 * 125 = 12500
K1T = NPC // K1M


def _build_k1():
    nc = bacc.Bacc("TRN2", target_bir_lowering=False)
    xt = nc.dram_tensor("xt", [D_IN, NPC], mybir.dt.float32, kind="ExternalInput")
    w = nc.dram_tensor("w", [D_IN, 3 * H], mybir.dt.float32, kind="ExternalInput")
    qkv = nc.dram_tensor("qkv", [NPC, 3 * H], mybir.dt.float32, kind="ExternalOutput")

    with tile.TileContext(nc) as tc:
        with ExitStack() as ctx:
            wp = ctx.enter_context(tc.tile_pool(name="wp", bufs=1))
            xp = ctx.enter_context(tc.tile_pool(name="xp", bufs=1))
            pp = ctx.enter_context(tc.tile_pool(name="pp", bufs=4, space="PSUM"))
            op = ctx.enter_context(tc.tile_pool(name="op", bufs=1))
            # whole X^T shard resident: partition p holds rows p and p+128
            xsb = xp.tile([P, 2, NPC], mybir.dt.float32, tag="xsb")
            nc.sync.dma_start(xsb[:], xt.rearrange("(g p) n -> p g n", g=2))
            w01 = wp.tile([P, 2, 3 * H], mybir.dt.float32, tag="w01")
            nc.sync.dma_start(w01[:], w.rearrange("(g p) e -> p g e", g=2))
            obuf = op.tile([K1M, K1T, 3 * H], mybir.dt.float32, tag="obuf")
            for t in range(K1T):
                r0 = t * K1M
                ps = pp.tile([K1M, 3 * H], mybir.dt.float32, tag="ps")
                nc.tensor.matmul(
                    ps[:], xsb[:, 0, r0 : r0 + K1M], w01[:, 0, :],
                    start=True, stop=False,
                )
                nc.tensor.matmul(
                    ps[:], xsb[:, 1, r0 : r0 + K1M], w01[:, 1, :],
                    start=False, stop=True,
                )
                nc.vector.tensor_copy(
                    obuf[:, t : t + 1, :].rearrange("p o e -> p (o e)"), ps[:]
                )
            nc.sync.dma_start(qkv.rearrange("(t p) e -> p t e", p=K1M), obuf[:])
    nc.compile()
    return nc


# ================================================================ kernel 2
def _build_k2(S):
    NV = S // 2
    nc = bacc.Bacc("TRN2", target_bir_lowering=False)
    kss = nc.dram_tensor("kss", [P, S, H], mybir.dt.float16, kind="ExternalInput")
    vss = nc.dram_tensor("vss", [P, S, H], mybir.dt.bfloat16, kind="ExternalInput")
    qvs = nc.dram_tensor("qvs", [P, NV, H], mybir.dt.float16, kind="ExternalInput")
    outp = nc.dram_tensor("outp", [P, NV, H + 1], mybir.dt.float32, kind="ExternalOutput")

    with tile.TileContext(nc) as tc:
        with ExitStack() as ctx:
            kp = ctx.enter_context(tc.tile_pool(name="kp", bufs=2))
            vp = ctx.enter_context(tc.tile_pool(name="vp", bufs=2))
            qp = ctx.enter_context(tc.tile_pool(name="qp", bufs=2))
            sp = ctx.enter_context(tc.tile_pool(name="sp", bufs=2))
            op = ctx.enter_context(tc.tile_pool(name="op", bufs=3))
            for a in range(0, S, B):
                w = min(B, S - a)
                nv2 = w // 2
                vb = a // 2
                kst = kp.tile([P, w, H], mybir.dt.float16, tag="kst")
                nc.sync.dma_start(kst[:], kss[:, a : a + w, :])
                vst = vp.tile([P, w, H], mybir.dt.bfloat16, tag="vst")
                nc.sync.dma_start(vst[:], vss[:, a : a + w, :])
                qvt = qp.tile([P, nv2, H], mybir.dt.float16, tag="qvt")
                nc.sync.dma_start(qvt[:], qvs[:, vb : vb + nv2, :])

                k4 = kst[:].rearrange("p (v t) e -> p v t e", t=2)
                v4 = vst[:].rearrange("p (v t) e -> p v t e", t=2)
                qv4 = qvt[:].rearrange("p v (o h) -> p v o h", o=1)

                pr = sp.tile([P, nv2, 2, H], mybir.dt.float16, tag="pr")
                nc.vector.tensor_tensor(
                    out=pr[:],
                    in0=qv4.to_broadcast([P, nv2, 2, H]),
                    in1=k4,
                    op=mybir.AluOpType.mult,
                )
                sc = sp.tile([P, nv2, 2], mybir.dt.float32, tag="sc")
                nc.vector.tensor_reduce(
                    out=sc[:], in_=pr[:], axis=mybir.AxisListType.X,
                    op=mybir.AluOpType.add,
                )
                ex = sp.tile([P, nv2, 2], mybir.dt.bfloat16, tag="ex")
                nc.scalar.activation(
                    ex[:], sc[:], mybir.ActivationFunctionType.Exp, scale=1.0 / DK
                )
                wv = sp.tile([P, nv2, 2, H], mybir.dt.bfloat16, tag="wv")
                nc.vector.tensor_tensor(
                    out=wv[:],
                    in0=ex[:].to_broadcast([P, nv2, 2, H]),
                    in1=v4,
                    op=mybir.AluOpType.mult,
                )
                pps = op.tile([P, nv2, H + 1], mybir.dt.float32, tag="pps")
                nc.vector.tensor_tensor(
                    out=pps[:, :, 0:H].rearrange("p v (o h) -> p v o h", o=1),
                    in0=wv[:, :, 0:1, :],
                    in1=wv[:, :, 1:2, :],
                    op=mybir.AluOpType.add,
                )
                nc.vector.tensor_tensor(
                    out=pps[:, :, H : H + 1],
                    in0=ex[:, :, 0:1],
                    in1=ex[:, :, 1:2],
                    op=mybir.AluOpType.add,
                )
                nc.sync.dma_start(outp[:, vb : vb + nv2, :], pps[:])
    nc.compile()
    return nc


# ================================================================ host build
def _build_streams(cc, S, Kh, Vb, Qloc):
    """Per-core slot streams: kss [P,S,32] fp16, vss [P,S,32] bf16,
    qvs [P,NV,32] fp16."""
    NV = S // 2
    slotdst = cc["slotdst"]
    qvnode = cc["qvnode"]
    real = slotdst >= 0

    kss = np.zeros((P, S, H), np.float16)
    kss[real] = Kh[slotdst[real]]
    vss = np.zeros((P, S, H), BF16)
    vss[real] = Vb[slotdst[real]]

    # pads sharing a pair with a real node: poison-K so exp(score) ~ 0
    qvn2 = np.repeat(qvnode, 2, axis=1)
    padm = (~real) & (qvn2 >= 0)
    if padm.any():
        q = Qloc[qvn2[padm]]
        kpad = (-200.0 / np.maximum((q * q).sum(1), 1e-9))[:, None] * q
        kss[padm] = kpad.astype(np.float16)

    qvs = np.zeros((P, NV, H), np.float16)
    validv = qvnode >= 0
    qvs[validv] = Qloc[qvnode[validv]].astype(np.float16)
    return kss, vss, qvs


def _combine(cc, outp):
    """Per-node segment reduction of pair partials; returns [NPC, H]."""
    qvnode = cc["qvnode"].ravel()
    w = outp.reshape(-1, H + 1)
    valid = qvnode >= 0
    idx = qvnode[valid]
    vals = w[valid]
    # vnodes of one node are contiguous (one partition, consecutive columns)
    starts = np.flatnonzero(np.diff(idx, prepend=idx[0] - 1) != 0)
    acc = np.add.reduceat(vals, starts, axis=0)
    out = np.zeros((NPC, H), np.float32)
    den = acc[:, H]
    den = np.where(den == 0, 1.0, den)
    out[idx[starts]] = acc[:, :H] / den[:, None]
    return out


# ================================================================ driver
def kernel(X, edge_index, Wq, Wk, Wv):
    X = np.ascontiguousarray(np.asarray(X, dtype=np.float32))
    Wq = np.asarray(Wq, dtype=np.float32)
    Wk = np.asarray(Wk, dtype=np.float32)
    Wv = np.asarray(Wv, dtype=np.float32)
    ei = np.asarray(edge_index)

    cores, S = _prep(ei)
    NV = S // 2

    # ---- kernel 1: projections
    if "k1" not in _cache:
        _cache["k1"] = _build_k1()
    k1 = _cache["k1"]
    w_cat = np.concatenate([Wq, Wk, Wv], axis=1).astype(np.float32)  # [256, 96]
    in1 = [
        {"xt": np.ascontiguousarray(X[c * NPC : (c + 1) * NPC].T), "w": w_cat}
        for c in range(NCORES)
    ]
    r1 = run_bass_kernel_spmd(k1, in1, core_ids=list(range(NCORES)))
    LAST_TIMES["k1"] = r1.exec_time_ns
    qkv = [r1.results[c]["qkv"] for c in range(NCORES)]
    Kf = np.concatenate([q[:, H : 2 * H] for q in qkv], axis=0)  # [N, 32]
    Vf = np.concatenate([q[:, 2 * H :] for q in qkv], axis=0)
    Kh = Kf.astype(np.float16)
    Vb = Vf.astype(BF16)

    # ---- kernel 2: stream slots, edge compute, pair partials
    if ("k2", S) not in _cache:
        _cache[("k2", S)] = _build_k2(S)
    k2 = _cache[("k2", S)]
    in2 = []
    for c in range(NCORES):
        kss, vss, qvs = _build_streams(cores[c], S, Kh, Vb, qkv[c][:, :H])
        in2.append({"kss": kss, "vss": vss, "qvs": qvs})
    r2 = run_bass_kernel_spmd(k2, in2, core_ids=list(range(NCORES)))
    LAST_TIMES["k2"] = r2.exec_time_ns

    # ---- host combine
    out = np.empty((N, H), dtype=np.float32)
    for c in range(NCORES):
        out[c * NPC : (c + 1) * NPC] = _combine(cores[c], r2.results[c]["outp"])
    return out


# revision 12
# speedup vs baseline: 1.0564x; 1.0564x over previous
"""Trainium2 Bass kernel for nn_MemoryAggregator (GNN attention aggregation).

Reference computation:
    Q = X@Wq; K = X@Wk; V = X@Wv            (X [100000,256], W [256,32])
    scores_e = <Q[src_e], K[dst_e]> / sqrt(32)   over 1.6M edges
    out[n]   = softmax-weighted sum over n's edges of V[dst_e]   ([100000,32])

Strategy (8 NeuronCores, SPMD, edges sharded by src node range):
  kernel1: per-core QKV projections of the core's 12500-node X shard (PE matmul).
  host:    arrange per-edge K|V rows and per-pair Q rows into flat per-partition
           slot streams (bf16).  Pad slots get K = -C*q/|q|^2 so their score is
           ~-35 and exp underflows to 0 -- no mask stream needed.
  kernel2: per core, stream slot blocks sequentially (no gathers) and compute
           scores -> exp -> pair partial sums [num(32) | den] on DVE/ACT.
  host:    per-node segment reduction of pair partials (contiguous runs,
           np.add.reduceat) + division.

Softmax max-subtraction is dropped: scores/sqrt(32) ~ N(0,4), max ~21, exp
safe in f32 (validated earlier at rel err ~3e-6; bf16 streams ~1e-3).
"""
import math
from contextlib import ExitStack

import numpy as np

import concourse.bass as bass
import concourse.tile as tile
from concourse import bacc, mybir
from concourse.bass_utils import run_bass_kernel_spmd

# ---------------------------------------------------------------- dimensions
N = 100000
E = 1600000
D_IN = 256
H = 32
DK = math.sqrt(H)
NCORES = 8
NPC = N // NCORES          # 12500 nodes per core
P = 128
B = 240                    # slot block width per k2 step (even)

BF16 = mybir.dt.np(mybir.dt.bfloat16)

_cache = {}
LAST_TIMES = {}


# ================================================================ host prep
def _prep_core(src_l, dst):
    order = np.argsort(src_l, kind="stable")
    dst_s = dst[order].astype(np.int32)

    d = np.bincount(src_l, minlength=NPC)
    v = (d + 1) // 2
    s = 2 * v

    # partition assignment: serpentine over nodes sorted by size desc
    node_order = np.argsort(-s, kind="stable")
    i = np.arange(NPC)
    pos = i % P
    pserp = np.where((i // P) % 2 == 0, pos, P - 1 - pos)
    part = np.empty(NPC, np.int64)
    part[node_order] = pserp
    load = np.bincount(part, weights=s, minlength=P).astype(np.int64)
    return {"d": d, "v": v, "s": s, "part": part, "dst_s": dst_s,
            "S_core": int(load.max())}


def _finalize_core(cc, S):
    NV = S // 2
    d, v, s, part = cc["d"], cc["v"], cc["s"], cc["part"]

    perm = np.lexsort((np.arange(NPC), part))
    part_sorted = part[perm]
    sizes = s[perm]
    cs = np.cumsum(sizes) - sizes
    pstart = np.searchsorted(part_sorted, np.arange(P))
    base_at = cs[np.minimum(pstart, NPC - 1)]
    within = cs - base_at[part_sorted]
    within_node = np.empty(NPC, np.int64)
    within_node[perm] = within

    slotdst = np.full((P, S), -1, np.int32)
    nodes_rep = np.repeat(np.arange(NPC), d)
    ranks = np.arange(int(d.sum())) - np.repeat(np.cumsum(d) - d, d)
    cols = within_node[nodes_rep] + ranks
    slotdst[part[nodes_rep], cols] = cc["dst_s"]

    qvnode = np.full((P, NV), -1, np.int32)
    vrep = np.repeat(np.arange(NPC), v)
    vranks = np.arange(int(v.sum())) - np.repeat(np.cumsum(v) - v, v)
    vcols = (within_node[vrep] >> 1) + vranks
    qvnode[part[vrep], vcols] = vrep

    cc["slotdst"] = slotdst
    cc["qvnode"] = qvnode
    del cc["dst_s"], cc["d"], cc["v"], cc["s"], cc["part"]


def _prep(edge_index):
    src = np.asarray(edge_index[0], dtype=np.int64)
    dst = np.asarray(edge_index[1], dtype=np.int64)
    core = src // NPC
    cores = []
    for c in range(NCORES):
        m = core == c
        cores.append(_prep_core(src[m] - c * NPC, dst[m]))
    S = max(cc["S_core"] for cc in cores)
    S = (S + 1) // 2 * 2
    for cc in cores:
        _finalize_core(cc, S)
    return cores, S


# ================================================================ kernel 1
K1M = 125                      # rows per matmul tile; 100 * 125 = 12500
K1T = NPC // K1M
K1CH = 4                       # xsb load chunks (overlap DMA with matmul)


def _build_k1():
    nc = bacc.Bacc("TRN2", target_bir_lowering=False)
    xt = nc.dram_tensor("xt", [D_IN, NPC], mybir.dt.float16, kind="ExternalInput")
    w = nc.dram_tensor("w", [D_IN, 3 * H], mybir.dt.float16, kind="ExternalInput")
    qkv = nc.dram_tensor("qkv", [NPC, 3 * H], mybir.dt.float16, kind="ExternalOutput")

    with tile.TileContext(nc) as tc:
        with ExitStack() as ctx:
            wp = ctx.enter_context(tc.tile_pool(name="wp", bufs=1))
            xp = ctx.enter_context(tc.tile_pool(name="xp", bufs=1))
            pp = ctx.enter_context(tc.tile_pool(name="pp", bufs=4, space="PSUM"))
            op = ctx.enter_context(tc.tile_pool(name="op", bufs=1))
            # whole X^T shard resident: partition p holds rows p and p+128
            xsb = xp.tile([P, 2, NPC], mybir.dt.float16, tag="xsb")
            csz = NPC // K1CH
            for ch in range(K1CH):
                nc.sync.dma_start(
                    xsb[:, :, ch * csz : (ch + 1) * csz],
                    xt[:, ch * csz : (ch + 1) * csz].rearrange(
                        "(g p) n -> p g n", g=2
                    ),
                )
            w01 = wp.tile([P, 2, 3 * H], mybir.dt.float16, tag="w01")
            nc.sync.dma_start(w01[:], w.rearrange("(g p) e -> p g e", g=2))
            obuf = op.tile([K1M, K1T, 3 * H], mybir.dt.float16, tag="obuf")
            for t in range(K1T):
                r0 = t * K1M
                ps = pp.tile([K1M, 3 * H], mybir.dt.float32, tag="ps")
                nc.tensor.matmul(
                    ps[:], xsb[:, 0, r0 : r0 + K1M], w01[:, 0, :],
                    start=True, stop=False,
                )
                nc.tensor.matmul(
                    ps[:], xsb[:, 1, r0 : r0 + K1M], w01[:, 1, :],
                    start=False, stop=True,
                )
                dst = obuf[:, t : t + 1, :].rearrange("p o e -> p (o e)")
                if t % 2 == 0:
                    nc.vector.tensor_copy(dst, ps[:])
                else:
                    nc.scalar.activation(
                        dst, ps[:], mybir.ActivationFunctionType.Copy
                    )
            nc.sync.dma_start(qkv.rearrange("(t p) e -> p t e", p=K1M), obuf[:])
    nc.compile()
    return nc


# ================================================================ kernel 2
def _build_k2(S):
    NV = S // 2
    nc = bacc.Bacc("TRN2", target_bir_lowering=False)
    kss = nc.dram_tensor("kss", [P, S, H], mybir.dt.float16, kind="ExternalInput")
    vsi = nc.dram_tensor("vsi", [P, NV, H, 2], mybir.dt.bfloat16, kind="ExternalInput")
    qvs = nc.dram_tensor("qvs", [P, NV, H], mybir.dt.float16, kind="ExternalInput")
    outn = nc.dram_tensor("outn", [P, NV, H], mybir.dt.bfloat16, kind="ExternalOutput")
    outd = nc.dram_tensor("outd", [P, NV, 2], mybir.dt.bfloat16, kind="ExternalOutput")

    with tile.TileContext(nc) as tc:
        with ExitStack() as ctx:
            kp = ctx.enter_context(tc.tile_pool(name="kp", bufs=2))
            vp = ctx.enter_context(tc.tile_pool(name="vp", bufs=2))
            qp = ctx.enter_context(tc.tile_pool(name="qp", bufs=2))
            sp = ctx.enter_context(tc.tile_pool(name="sp", bufs=2))
            op = ctx.enter_context(tc.tile_pool(name="op", bufs=3))
            for a in range(0, S, B):
                w = min(B, S - a)
                nv2 = w // 2
                vb = a // 2
                kst = kp.tile([P, w, H], mybir.dt.float16, tag="kst")
                nc.sync.dma_start(kst[:], kss[:, a : a + w, :])
                vst = vp.tile([P, nv2, H, 2], mybir.dt.bfloat16, tag="vst")
                nc.sync.dma_start(vst[:], vsi[:, vb : vb + nv2, :, :])
                qvt = qp.tile([P, nv2, H], mybir.dt.float16, tag="qvt")
                nc.sync.dma_start(qvt[:], qvs[:, vb : vb + nv2, :])

                k4 = kst[:].rearrange("p (v t) e -> p v t e", t=2)
                qv4 = qvt[:].rearrange("p v (o h) -> p v o h", o=1)

                # scores on DVE: q*k -> half-add -> reduce16
                pr = sp.tile([P, nv2, 2, H], mybir.dt.float16, tag="pr")
                nc.vector.tensor_tensor(
                    out=pr[:],
                    in0=qv4.to_broadcast([P, nv2, 2, H]),
                    in1=k4,
                    op=mybir.AluOpType.mult,
                )
                ph = sp.tile([P, nv2, 2, H // 2], mybir.dt.float16, tag="ph")
                nc.vector.tensor_tensor(
                    out=ph[:],
                    in0=pr[:, :, :, 0 : H // 2],
                    in1=pr[:, :, :, H // 2 : H],
                    op=mybir.AluOpType.add,
                )
                sc = sp.tile([P, nv2, 2], mybir.dt.float16, tag="sc")
                with nc.allow_low_precision(reason="fp16 scores, |s|<70"):
                    nc.vector.tensor_reduce(
                        out=sc[:], in_=ph[:], axis=mybir.AxisListType.X,
                        op=mybir.AluOpType.add,
                    )
                # exp on ACT
                ex = sp.tile([P, nv2, 2], mybir.dt.bfloat16, tag="ex")
                nc.scalar.activation(
                    ex[:], sc[:], mybir.ActivationFunctionType.Exp, scale=1.0 / DK
                )
                # weighted V + pair sum on GpSimd
                exb = (
                    ex[:]
                    .rearrange("p v (o t) -> p v o t", o=1)
                    .to_broadcast([P, nv2, H, 2])
                )
                wv = sp.tile([P, nv2, H, 2], mybir.dt.bfloat16, tag="wv")
                nc.gpsimd.tensor_tensor(
                    out=wv[:], in0=exb, in1=vst[:], op=mybir.AluOpType.mult
                )
                non = op.tile([P, nv2, H], mybir.dt.bfloat16, tag="non")
                nc.gpsimd.tensor_tensor(
                    out=non[:].rearrange("p v (o h) -> p v o h", o=1),
                    in0=wv[:, :, :, 0:1].rearrange("p v h t -> p v t h"),
                    in1=wv[:, :, :, 1:2].rearrange("p v h t -> p v t h"),
                    op=mybir.AluOpType.add,
                )
                nc.sync.dma_start(outn[:, vb : vb + nv2, :], non[:])
                nc.sync.dma_start(outd[:, vb : vb + nv2, :], ex[:])
    nc.compile()
    return nc


# ================================================================ host build
def _build_streams(cc, S, Kh, Vb, Qloc):
    """Per-core slot streams: kss [P,S,32] fp16, vsi [P,NV,32,2] bf16
    (pair-interleaved), qvs [P,NV,32] fp16."""
    NV = S // 2
    slotdst = cc["slotdst"]
    qvnode = cc["qvnode"]
    real = slotdst >= 0

    kss = np.zeros((P, S, H), np.float16)
    kss[real] = Kh[slotdst[real]]
    vss = np.zeros((P, S, H), BF16)
    vss[real] = Vb[slotdst[real]]
    vsi = np.ascontiguousarray(
        vss.reshape(P, NV, 2, H).transpose(0, 1, 3, 2)
    )

    # pads sharing a pair with a real node: poison-K so exp(score) ~ 0
    qvn2 = np.repeat(qvnode, 2, axis=1)
    padm = (~real) & (qvn2 >= 0)
    if padm.any():
        q = Qloc[qvn2[padm]].astype(np.float32)
        kpad = (-200.0 / np.maximum((q * q).sum(1), 1e-9))[:, None] * q
        kss[padm] = kpad.astype(np.float16)

    qvs = np.zeros((P, NV, H), np.float16)
    validv = qvnode >= 0
    qvs[validv] = Qloc[qvnode[validv]]
    return kss, vsi, qvs


def _combine(cc, outn, outd):
    """Per-node segment reduction of pair partials; returns [NPC, H]."""
    qvnode = cc["qvnode"].ravel()
    valid = qvnode >= 0
    idx = qvnode[valid]
    num = outn.reshape(-1, H)[valid].astype(np.float32)
    den = outd.reshape(-1, 2)[valid].astype(np.float32).sum(1)
    # vnodes of one node are contiguous (one partition, consecutive columns)
    starts = np.flatnonzero(np.diff(idx, prepend=idx[0] - 1) != 0)
    accn = np.add.reduceat(num, starts, axis=0)
    accd = np.add.reduceat(den, starts)
    accd = np.where(accd == 0, 1.0, accd)
    out = np.zeros((NPC, H), np.float32)
    out[idx[starts]] = accn / accd[:, None]
    return out


# ================================================================ driver
def kernel(X, edge_index, Wq, Wk, Wv):
    X = np.ascontiguousarray(np.asarray(X, dtype=np.float32))
    Wq = np.asarray(Wq, dtype=np.float32)
    Wk = np.asarray(Wk, dtype=np.float32)
    Wv = np.asarray(Wv, dtype=np.float32)
    ei = np.asarray(edge_index)

    cores, S = _prep(ei)
    NV = S // 2

    # ---- kernel 1: projections
    if "k1" not in _cache:
        _cache["k1"] = _build_k1()
    k1 = _cache["k1"]
    w_cat = np.concatenate([Wq, Wk, Wv], axis=1).astype(np.float16)  # [256, 96]
    in1 = [
        {
            "xt": np.ascontiguousarray(X[c * NPC : (c + 1) * NPC].T).astype(
                np.float16
            ),
            "w": w_cat,
        }
        for c in range(NCORES)
    ]
    r1 = run_bass_kernel_spmd(k1, in1, core_ids=list(range(NCORES)))
    LAST_TIMES["k1"] = r1.exec_time_ns
    qkv = [r1.results[c]["qkv"] for c in range(NCORES)]
    Kh = np.ascontiguousarray(
        np.concatenate([q[:, H : 2 * H] for q in qkv], axis=0)
    )  # [N, 32] fp16
    Vb = np.concatenate([q[:, 2 * H :] for q in qkv], axis=0).astype(BF16)

    # ---- kernel 2: stream slots, edge compute, pair partials
    if ("k2", S) not in _cache:
        _cache[("k2", S)] = _build_k2(S)
    k2 = _cache[("k2", S)]
    in2 = []
    for c in range(NCORES):
        kss, vsi, qvs = _build_streams(cores[c], S, Kh, Vb, qkv[c][:, :H])
        in2.append({"kss": kss, "vsi": vsi, "qvs": qvs})
    r2 = run_bass_kernel_spmd(k2, in2, core_ids=list(range(NCORES)))
    LAST_TIMES["k2"] = r2.exec_time_ns

    # ---- host combine
    out = np.empty((N, H), dtype=np.float32)
    for c in range(NCORES):
        out[c * NPC : (c + 1) * NPC] = _combine(
            cores[c], r2.results[c]["outn"], r2.results[c]["outd"]
        )
    return out


# revision 25
# speedup vs baseline: 17.9106x; 16.9540x over previous
"""Trainium2 Bass kernel for nn_MemoryAggregator (GNN attention aggregation).

Reference computation:
    Q = X@Wq; K = X@Wk; V = X@Wv            (X [100000,256], W [256,32])
    scores_e = <Q[src_e], K[dst_e]> / sqrt(32)   over 1.6M edges
    out[n]   = softmax-weighted sum over n's edges of V[dst_e]   ([100000,32])

Strategy (8 NeuronCores, SPMD, edges sharded by src node range):
  kernel1: per-core QKV projections of the core's 12500-node X shard (PE matmul).
  host:    arrange per-edge K|V rows and per-pair Q rows into flat per-partition
           slot streams (bf16).  Pad slots get K = -C*q/|q|^2 so their score is
           ~-35 and exp underflows to 0 -- no mask stream needed.
  kernel2: per core, stream slot blocks sequentially (no gathers) and compute
           scores -> exp -> pair partial sums [num(32) | den] on DVE/ACT.
  host:    per-node segment reduction of pair partials (contiguous runs,
           np.add.reduceat) + division.

Softmax max-subtraction is dropped: scores/sqrt(32) ~ N(0,4), max ~21, exp
safe in f32 (validated earlier at rel err ~3e-6; bf16 streams ~1e-3).
"""
import math
from contextlib import ExitStack

import numpy as np

import concourse.bass as bass
import concourse.tile as tile
from concourse import bacc, mybir
from concourse.bass_utils import run_bass_kernel_spmd

# ---------------------------------------------------------------- dimensions
N = 100000
E = 1600000
D_IN = 256
H = 32
DK = math.sqrt(H)
NCORES = 8
NPC = N // NCORES          # 12500 nodes per core
P = 128
B = 240                    # slot block width per k2 step (even)

BF16 = mybir.dt.np(mybir.dt.bfloat16)

_cache = {}
LAST_TIMES = {}
LAST_S = None


# ================================================================ host prep
def _prep_core(src_l, dst):
    order = np.argsort(src_l, kind="stable")
    dst_s = dst[order].astype(np.int32)

    d = np.bincount(src_l, minlength=NPC)
    v = (d + 1) // 2
    s = 2 * v

    # partition assignment: serpentine over nodes sorted by size desc
    node_order = np.argsort(-s, kind="stable")
    i = np.arange(NPC)
    pos = i % P
    pserp = np.where((i // P) % 2 == 0, pos, P - 1 - pos)
    part = np.empty(NPC, np.int64)
    part[node_order] = pserp
    load = np.bincount(part, weights=s, minlength=P).astype(np.int64)
    return {"d": d, "v": v, "s": s, "part": part, "dst_s": dst_s,
            "S_core": int(load.max())}


def _finalize_core(cc, S):
    NV = S // 2
    d, v, s, part = cc["d"], cc["v"], cc["s"], cc["part"]

    perm = np.lexsort((np.arange(NPC), part))
    part_sorted = part[perm]
    sizes = s[perm]
    cs = np.cumsum(sizes) - sizes
    pstart = np.searchsorted(part_sorted, np.arange(P))
    base_at = cs[np.minimum(pstart, NPC - 1)]
    within = cs - base_at[part_sorted]
    within_node = np.empty(NPC, np.int64)
    within_node[perm] = within

    slotdst = np.full((P, S), -1, np.int32)
    nodes_rep = np.repeat(np.arange(NPC), d)
    ranks = np.arange(int(d.sum())) - np.repeat(np.cumsum(d) - d, d)
    cols = within_node[nodes_rep] + ranks
    slotdst[part[nodes_rep], cols] = cc["dst_s"]

    qvnode = np.full((P, NV), -1, np.int32)
    vrep = np.repeat(np.arange(NPC), v)
    vranks = np.arange(int(v.sum())) - np.repeat(np.cumsum(v) - v, v)
    vcols = (within_node[vrep] >> 1) + vranks
    qvnode[part[vrep], vcols] = vrep

    cc["slotdst"] = slotdst
    cc["qvnode"] = qvnode
    del cc["dst_s"], cc["d"], cc["v"], cc["s"], cc["part"]


def _prep(edge_index):
    src = np.asarray(edge_index[0], dtype=np.int64)
    dst = np.asarray(edge_index[1], dtype=np.int64)
    core = src // NPC
    cores = []
    for c in range(NCORES):
        m = core == c
        cores.append(_prep_core(src[m] - c * NPC, dst[m]))
    S = max(cc["S_core"] for cc in cores)
    S = (S + 1) // 2 * 2
    for cc in cores:
        _finalize_core(cc, S)
    return cores, S


# ================================================================ kernel 1
K1M = 125                      # rows per matmul tile; 100 * 125 = 12500
K1T = NPC // K1M
K1CH = 4                       # xsb load chunks (overlap DMA with matmul)


def _build_k1(reps=1, bench_outs=False):
    OR = reps if bench_outs else 1
    nc = bacc.Bacc("TRN2", target_bir_lowering=False)
    xt = nc.dram_tensor("xt", [D_IN, NPC], mybir.dt.float16, kind="ExternalInput")
    w = nc.dram_tensor("w", [D_IN, 3 * H], mybir.dt.float16, kind="ExternalInput")
    qkv = nc.dram_tensor(
        "qkv", [OR * NPC, 3 * H], mybir.dt.float16, kind="ExternalOutput"
    )

    with tile.TileContext(nc) as tc:
        with ExitStack() as ctx:
            wp = ctx.enter_context(tc.tile_pool(name="wp", bufs=1))
            xp = ctx.enter_context(tc.tile_pool(name="xp", bufs=2))
            pp = ctx.enter_context(tc.tile_pool(name="pp", bufs=4, space="PSUM"))
            op = ctx.enter_context(tc.tile_pool(name="op", bufs=2))
            w01 = wp.tile([P, 2, 3 * H], mybir.dt.float16, tag="w01")
            nc.sync.dma_start(w01[:], w.rearrange("(g p) e -> p g e", g=2))
            for rep in range(reps):
                # whole X^T shard resident: partition p holds rows p, p+128
                xsb = xp.tile([P, 2, NPC], mybir.dt.float16, tag="xsb")
                csz = NPC // K1CH
                for ch in range(K1CH):
                    nc.sync.dma_start(
                        xsb[:, :, ch * csz : (ch + 1) * csz],
                        xt[:, ch * csz : (ch + 1) * csz].rearrange(
                            "(g p) n -> p g n", g=2
                        ),
                    )
                obuf = op.tile([K1M, K1T, 3 * H], mybir.dt.float16, tag="obuf")
                for t in range(K1T):
                    r0 = t * K1M
                    ps = pp.tile([K1M, 3 * H], mybir.dt.float32, tag="ps")
                    nc.tensor.matmul(
                        ps[:], xsb[:, 0, r0 : r0 + K1M], w01[:, 0, :],
                        start=True, stop=False,
                    )
                    nc.tensor.matmul(
                        ps[:], xsb[:, 1, r0 : r0 + K1M], w01[:, 1, :],
                        start=False, stop=True,
                    )
                    dst = obuf[:, t : t + 1, :].rearrange("p o e -> p (o e)")
                    if t % 2 == 0:
                        nc.vector.tensor_copy(dst, ps[:])
                    else:
                        nc.scalar.activation(
                            dst, ps[:], mybir.ActivationFunctionType.Copy
                        )
                o0 = (rep * NPC if bench_outs else 0)
                nc.sync.dma_start(
                    qkv[o0 : o0 + NPC, :].rearrange("(t p) e -> p t e", p=K1M),
                    obuf[:],
                )
    nc.compile()
    return nc


# ================================================================ kernel 2
def _build_k2(S, reps=1, wv_eng="vector", num_eng="vector", bench_outs=False):
    NV = S // 2
    OR = reps if bench_outs else 1  # distinct live output region per rep
    nc = bacc.Bacc("TRN2", target_bir_lowering=False)
    kss = nc.dram_tensor("kss", [P, S, H], mybir.dt.float16, kind="ExternalInput")
    vsi = nc.dram_tensor("vsi", [P, NV, H, 2], mybir.dt.bfloat16, kind="ExternalInput")
    qvs = nc.dram_tensor("qvs", [P, NV, H], mybir.dt.float16, kind="ExternalInput")
    outn = nc.dram_tensor("outn", [P, OR * NV, H], mybir.dt.bfloat16, kind="ExternalOutput")
    outd = nc.dram_tensor("outd", [P, OR * NV, 2], mybir.dt.bfloat16, kind="ExternalOutput")

    with tile.TileContext(nc) as tc:
        with ExitStack() as ctx:
            kp = ctx.enter_context(tc.tile_pool(name="kp", bufs=2))
            vp = ctx.enter_context(tc.tile_pool(name="vp", bufs=2))
            qp = ctx.enter_context(tc.tile_pool(name="qp", bufs=2))
            sp = ctx.enter_context(tc.tile_pool(name="sp", bufs=2))
            op = ctx.enter_context(tc.tile_pool(name="op", bufs=3))
            for rep, a in [
                (r, a) for r in range(reps) for a in range(0, S, B)
            ]:
                w = min(B, S - a)
                nv2 = w // 2
                vb = a // 2
                ob = (rep * NV if bench_outs else 0) + vb
                kst = kp.tile([P, w, H], mybir.dt.float16, tag="kst")
                nc.sync.dma_start(kst[:], kss[:, a : a + w, :])
                vst = vp.tile([P, nv2, H, 2], mybir.dt.bfloat16, tag="vst")
                nc.sync.dma_start(vst[:], vsi[:, vb : vb + nv2, :, :])
                qvt = qp.tile([P, nv2, H], mybir.dt.float16, tag="qvt")
                nc.sync.dma_start(qvt[:], qvs[:, vb : vb + nv2, :])

                k4 = kst[:].rearrange("p (v t) e -> p v t e", t=2)
                qv4 = qvt[:].rearrange("p v (o h) -> p v o h", o=1)

                # scores on DVE: q*k -> half-add -> reduce16
                pr = sp.tile([P, nv2, 2, H], mybir.dt.float16, tag="pr")
                nc.vector.tensor_tensor(
                    out=pr[:],
                    in0=qv4.to_broadcast([P, nv2, 2, H]),
                    in1=k4,
                    op=mybir.AluOpType.mult,
                )
                ph = sp.tile([P, nv2, 2, H // 2], mybir.dt.float16, tag="ph")
                nc.vector.tensor_tensor(
                    out=ph[:],
                    in0=pr[:, :, :, 0 : H // 2],
                    in1=pr[:, :, :, H // 2 : H],
                    op=mybir.AluOpType.add,
                )
                sc = sp.tile([P, nv2, 2], mybir.dt.float16, tag="sc")
                with nc.allow_low_precision(reason="fp16 scores, |s|<70"):
                    nc.vector.tensor_reduce(
                        out=sc[:], in_=ph[:], axis=mybir.AxisListType.X,
                        op=mybir.AluOpType.add,
                    )
                # exp on ACT
                ex = sp.tile([P, nv2, 2], mybir.dt.bfloat16, tag="ex")
                nc.scalar.activation(
                    ex[:], sc[:], mybir.ActivationFunctionType.Exp, scale=1.0 / DK
                )
                # weighted V + pair sum on GpSimd
                exb = (
                    ex[:]
                    .rearrange("p v (o t) -> p v o t", o=1)
                    .to_broadcast([P, nv2, H, 2])
                )
                wv = sp.tile([P, nv2, H, 2], mybir.dt.bfloat16, tag="wv")
                (nc.gpsimd if wv_eng == "pool" else nc.vector).tensor_tensor(
                    out=wv[:], in0=exb, in1=vst[:], op=mybir.AluOpType.mult
                )
                non = op.tile([P, nv2, H], mybir.dt.bfloat16, tag="non")
                (nc.gpsimd if num_eng == "pool" else nc.vector).tensor_tensor(
                    out=non[:].rearrange("p v (o h) -> p v o h", o=1),
                    in0=wv[:, :, :, 0:1].rearrange("p v h t -> p v t h"),
                    in1=wv[:, :, :, 1:2].rearrange("p v h t -> p v t h"),
                    op=mybir.AluOpType.add,
                )
                nc.sync.dma_start(outn[:, ob : ob + nv2, :], non[:])
                nc.sync.dma_start(outd[:, ob : ob + nv2, :], ex[:])
    nc.compile()
    return nc


# ================================================================ host build
def _build_streams(cc, S, Kh, Vb, Qloc):
    """Per-core slot streams: kss [P,S,32] fp16, vsi [P,NV,32,2] bf16
    (pair-interleaved), qvs [P,NV,32] fp16."""
    NV = S // 2
    slotdst = cc["slotdst"]
    qvnode = cc["qvnode"]
    real = slotdst >= 0

    kss = np.zeros((P, S, H), np.float16)
    kss[real] = Kh[slotdst[real]]
    vss = np.zeros((P, S, H), BF16)
    vss[real] = Vb[slotdst[real]]
    vsi = np.ascontiguousarray(
        vss.reshape(P, NV, 2, H).transpose(0, 1, 3, 2)
    )

    # pads sharing a pair with a real node: poison-K so exp(score) ~ 0
    qvn2 = np.repeat(qvnode, 2, axis=1)
    padm = (~real) & (qvn2 >= 0)
    if padm.any():
        q = Qloc[qvn2[padm]].astype(np.float32)
        kpad = (-200.0 / np.maximum((q * q).sum(1), 1e-9))[:, None] * q
        kss[padm] = kpad.astype(np.float16)

    qvs = np.zeros((P, NV, H), np.float16)
    validv = qvnode >= 0
    qvs[validv] = Qloc[qvnode[validv]]
    return kss, vsi, qvs


def _combine(cc, outn, outd):
    """Per-node segment reduction of pair partials; returns [NPC, H]."""
    qvnode = cc["qvnode"].ravel()
    valid = qvnode >= 0
    idx = qvnode[valid]
    num = outn.reshape(-1, H)[valid].astype(np.float32)
    den = outd.reshape(-1, 2)[valid].astype(np.float32).sum(1)
    # vnodes of one node are contiguous (one partition, consecutive columns)
    starts = np.flatnonzero(np.diff(idx, prepend=idx[0] - 1) != 0)
    accn = np.add.reduceat(num, starts, axis=0)
    accd = np.add.reduceat(den, starts)
    accd = np.where(accd == 0, 1.0, accd)
    out = np.zeros((NPC, H), np.float32)
    out[idx[starts]] = accn / accd[:, None]
    return out


# ================================================================ driver
def kernel(X, edge_index, Wq, Wk, Wv):
    X = np.ascontiguousarray(np.asarray(X, dtype=np.float32))
    Wq = np.asarray(Wq, dtype=np.float32)
    Wk = np.asarray(Wk, dtype=np.float32)
    Wv = np.asarray(Wv, dtype=np.float32)
    ei = np.asarray(edge_index)

    global LAST_S
    cores, S = _prep(ei)
    LAST_S = S
    NV = S // 2

    # ---- kernel 1: projections
    if "k1" not in _cache:
        _cache["k1"] = _build_k1()
    k1 = _cache["k1"]
    w_cat = np.concatenate([Wq, Wk, Wv], axis=1).astype(np.float16)  # [256, 96]
    in1 = [
        {
            "xt": np.ascontiguousarray(X[c * NPC : (c + 1) * NPC].T).astype(
                np.float16
            ),
            "w": w_cat,
        }
        for c in range(NCORES)
    ]
    r1 = run_bass_kernel_spmd(k1, in1, core_ids=list(range(NCORES)))
    LAST_TIMES["k1"] = r1.exec_time_ns
    qkv = [r1.results[c]["qkv"] for c in range(NCORES)]
    Kh = np.ascontiguousarray(
        np.concatenate([q[:, H : 2 * H] for q in qkv], axis=0)
    )  # [N, 32] fp16
    Vb = np.concatenate([q[:, 2 * H :] for q in qkv], axis=0).astype(BF16)

    # ---- kernel 2: stream slots, edge compute, pair partials
    if ("k2", S) not in _cache:
        _cache[("k2", S)] = _build_k2(S)
    k2 = _cache[("k2", S)]
    in2 = []
    for c in range(NCORES):
        kss, vsi, qvs = _build_streams(cores[c], S, Kh, Vb, qkv[c][:, :H])
        in2.append({"kss": kss, "vsi": vsi, "qvs": qvs})
    r2 = run_bass_kernel_spmd(k2, in2, core_ids=list(range(NCORES)))
    LAST_TIMES["k2"] = r2.exec_time_ns

    # ---- host combine
    out = np.empty((N, H), dtype=np.float32)
    for c in range(NCORES):
        out[c * NPC : (c + 1) * NPC] = _combine(
            cores[c], r2.results[c]["outn"], r2.results[c]["outd"]
        )
    return out
